# revision 51
# baseline (speedup 1.0000x reference)
"""Trainium2 Bass kernel for nn_AutoEncoder_75282186764670.

Set autoencoder over ragged segments (B=32768 sets, N=131072 elements,
DIM=128, HID=64, MAXP=17).  Data-parallel over sets: each of the 8 cores
owns 4096 consecutive segment ids and their contiguous x rows.

Per-core pipeline:
  A1 per 128-element tile (batched 4/DMA): psumA = x @ [psi_w1|rank_w|rowmean]
     -> centered d rows (kept in SBUF), sum-of-squares, mag column
  A2 batched sqrt/reciprocal for all LayerNorm rstd; mag relayout to flat
     element order via PE transpose
  A3 normalize+tanh -> psi = th @ [psi_w2;psi_b2] -> psi_dram (batched)
  B  ranks: halo layout [128, 132+2*16]; rank_i = #{j in seg: mag_j < mag_i}
     + #{j<i in seg: mag_j == mag_i} via shifted compares (segments <= 16)
  C  y1 = psi * pe_enc[rank] via transposed one-hot matmul against the
     pe table; y1 rows + seg id + ones -> y1_dram [Npad+CAP, 66] (batched)
  W  per 128-segment window: ONE dynamic-offset DMA pulls its <=768
     element rows; one-hot(seg - 128w) matmul accumulate -> y2, counts n
  Z  z = phi(concat(y2, onehot(n))), logits = size(z), n_dec = argmax,
     mask rows -> DRAM, zT kept in SBUF
  D  decoder (bf16 matmuls): hT = tanh(w1.T @ (zT*pe_dec_T[:,p]) + b1),
     xr = [hT;1].T @ [w2;b2], masked by (p < n_dec), rows (b*17+p).

Notes:
 - rank_b is dropped: a constant shift never changes within-segment order.
 - psi_b1 enters LN only as (b1 - mean(b1)) added after mean-subtraction;
   the row-mean of x@w1 comes from an extra matmul column (w1 @ 1/96).
 - one_hot(n) always sums to 1 here (n <= 16 asserted), so phi_b1 is folded
   into the one-hot weight table.
 - harness pre-zeroes ExternalOutput buffers.
"""

import sys

import numpy as np

for _p in ("/opt/trn_rl_repo",):
    if _p not in sys.path:
        sys.path.insert(0, _p)

import ml_dtypes

import concourse.bacc as bacc
import concourse.bass as bass
import concourse.mybir as mybir
import concourse.tile as tile
from concourse.bass_utils import run_bass_kernel_spmd

F32 = mybir.dt.float32
BF16 = mybir.dt.bfloat16
I32 = mybir.dt.int32
AF = mybir.ActivationFunctionType
OP = mybir.AluOpType
AX = mybir.AxisListType

P = 128
DIM = 128
HID = 64
MAXP = 17
NCORES = 8
B = 32768
N = 131072
BLOC = B // NCORES          # segments per core
NPAD = 16896                # element capacity per core (132 * 128)
C = NPAD // P               # 132 columns in halo layout
NT = NPAD // P              # 132 element tiles
Q = 4                       # element tiles per DMA batch
NK = NT // Q                # 33
HALO = 16
DMAX = 15                   # max within-segment distance (seg size <= 16)
SEGW = 128                  # segments per window
NW = BLOC // SEGW           # 32 windows
CAP = 768                   # element capacity per window
NCHUNK = CAP // P           # 6
Y1ROWS = NPAD + CAP
SEGH_LEN = HALO + NPAD + HALO + C   # extra C so right-halo rearrange stays in bounds
BIGSEG = 1.0e6
EPS = 1e-5

BF16_D = True
DYN_GATHER = True
STOP_AFTER = None  # 'A'|'B'|'C'|'WZ' truncates the program for HW bisection
_CACHED = {}
_STAGE_MARKS = []   # (stage_name, first_inst_id) in build order

# All replicated f32 constants packed into one [128, X] tensor (one DMA).
_CONST_SPECS = [
    ("w1m", 128, 98), ("bdiff", 128, 96), ("psi_g", 128, 96), ("psi_bt", 128, 96),
    ("psi_w2e", 97, 64),
    ("ident", 128, 128), ("eye17", 17, 17), ("ones_row", 1, 128), ("ones_col", 128, 1),
    ("pe_e_w1", 17, 40), ("pe_e_b1", 1, 40), ("pe_e_g", 17, 40), ("pe_e_bt", 17, 40),
    ("pe_e_w2e", 41, 64),
    ("pe_d_w1", 17, 40), ("pe_d_b1", 1, 40), ("pe_d_g", 17, 40), ("pe_d_bt", 17, 40),
    ("pe_d_w2e", 41, 64),
    ("phi_w1a", 64, 72), ("phi_w1b", 17, 72), ("phi_w2e", 73, 64),
    ("size_w1e", 65, 40), ("size_w2e", 41, 17),
    ("dec_b1c", 96, 1),
    ("iota17c", 17, 1), ("iotaF", 128, 128), ("chunkio", 128, 6),
    ("rev17", 128, 17), ("iota17r", 128, 17),
]
_CONST_OFF = {}
_off = 0
for _n, _r, _c in _CONST_SPECS:
    _CONST_OFF[_n] = (_off, _r, _c)
    _off += _c
CPCOLS = _off

# bf16 decoder weights packed the same way.
_CONST16_SPECS = [("dec_w1", 64, 96), ("dec_w2e", 97, 128)]
_CONST16_OFF = {}
_off = 0
for _n, _r, _c in _CONST16_SPECS:
    _CONST16_OFF[_n] = (_off, _r, _c)
    _off += _c
CP16COLS = _off


def _build_bass():
    nc = bacc.Bacc("TRN2", target_bir_lowering=False, debug=False)
    _STAGE_MARKS.clear()

    def mark(name):
        _STAGE_MARKS.append((name, nc.next_id()))

    # per-core (sharded) inputs
    xT = nc.declare_dram_parameter("xT", [P, NPAD], F32, isOutput=False)
    segh = nc.declare_dram_parameter("segh", [SEGH_LEN], F32, isOutput=False)
    cpack = nc.declare_dram_parameter("cpack", [P, CPCOLS], F32, isOutput=False)
    DTD = BF16 if BF16_D else F32
    cpack16 = nc.declare_dram_parameter("cpack16", [P, CP16COLS], DTD, isOutput=False)

    out_xr = nc.declare_dram_parameter("out_xr", [BLOC * MAXP, DIM], F32, isOutput=True)
    out_mask = nc.declare_dram_parameter("out_mask", [BLOC, MAXP], F32, isOutput=True)

    # internal DRAM scratch
    mag_h = nc.dram_tensor("mag_h", [SEGH_LEN], F32)
    rankf = nc.dram_tensor("rankf", [NPAD], F32)
    psi_dram = nc.dram_tensor("psi_dram", [NPAD, 64], F32)
    y1_dram = nc.dram_tensor("y1_dram", [Y1ROWS, 66], F32)

    with tile.TileContext(nc) as tc:
        import contextlib

        with contextlib.ExitStack() as ctx:
            consts = ctx.enter_context(tc.tile_pool(name="consts", bufs=1))
            persist = ctx.enter_context(tc.tile_pool(name="persist", bufs=1))
            work = ctx.enter_context(tc.tile_pool(name="work", bufs=3))
            ppA = ctx.enter_context(tc.tile_pool(name="ppA", bufs=4, space="PSUM"))
            ppT = ctx.enter_context(tc.tile_pool(name="ppT", bufs=4, space="PSUM"))

            # ---- constants: one DMA each ----
            cbuf = consts.tile([P, CPCOLS], F32, name="cbuf")
            nc.sync.dma_start(out=cbuf[:], in_=cpack[:, :])
            cbuf16 = consts.tile([P, CP16COLS], DTD, name="cbuf16")
            nc.sync.dma_start(out=cbuf16[:], in_=cpack16[:, :])

            def CS(name):
                off, r, c = _CONST_OFF[name]
                return cbuf[0:r, off:off + c]

            def CS16(name):
                off, r, c = _CONST16_OFF[name]
                return cbuf16[0:r, off:off + c]

            c_w1m = CS("w1m")
            c_bdiff = CS("bdiff")
            c_psig = CS("psi_g")
            c_psibt = CS("psi_bt")
            c_psiw2e = CS("psi_w2e")
            c_ident = CS("ident")
            c_eye17 = CS("eye17")
            c_ones_row = CS("ones_row")
            c_ones_col = CS("ones_col")
            c_pew1 = {t: CS(f"pe_{t}_w1") for t in ("e", "d")}
            c_peb1 = {t: CS(f"pe_{t}_b1") for t in ("e", "d")}
            c_peg = {t: CS(f"pe_{t}_g") for t in ("e", "d")}
            c_pebt = {t: CS(f"pe_{t}_bt") for t in ("e", "d")}
            c_pew2e = {t: CS(f"pe_{t}_w2e") for t in ("e", "d")}
            c_phiw1a = CS("phi_w1a")
            c_phiw1b = CS("phi_w1b")
            c_phiw2e = CS("phi_w2e")
            c_sizew1e = CS("size_w1e")
            c_sizew2e = CS("size_w2e")
            c_decb1c = CS("dec_b1c")
            c_iota17c = CS("iota17c")
            c_iotaF = CS("iotaF")
            c_rev17 = CS("rev17")
            c_iota17r = CS("iota17r")
            c16_decw1 = CS16("dec_w1")
            c16_decw2e = CS16("dec_w2e")

            # persistent SBUF state
            zTbuf = persist.tile([65, BLOC], F32, name="zTbuf")      # row 64 = ones
            zTbuf16 = persist.tile([64, BLOC], DTD, name="zTbuf16")
            mask_buf = persist.tile([P, NW * MAXP], F32, name="mask_buf")
            dbuf = persist.tile([P, NT, 96], F32, name="dbuf")
            ssqbuf = persist.tile([P, NT], F32, name="ssqbuf")
            rstdbuf = persist.tile([P, NT], F32, name="rstdbuf")
            magbuf = persist.tile([P, NT], F32, name="magbuf")
            segbufT = persist.tile([P, NT], F32, name="segbufT")
            acc32 = persist.tile([P, NW], F32, name="acc32")
            oroi = persist.tile([1, NW], I32, name="oroi")
            orof = persist.tile([1, NW], F32, name="orof")
            nc.gpsimd.memset(zTbuf[64:65, :], 1.0)
            c_eps = persist.tile([P, 1], F32, name="c_eps")
            nc.gpsimd.memset(c_eps[:], EPS)

            # zero mag_h halo edges (never written by stage A2, read by stage B)
            zedge = persist.tile([1, C + 2 * HALO], F32, name="zedge")
            nc.vector.memset(zedge[:], 0.0)
            nc.sync.dma_start(out=mag_h[0:HALO].unsqueeze(0), in_=zedge[0:1, 0:HALO])
            nc.sync.dma_start(out=mag_h[HALO + NPAD:SEGH_LEN].unsqueeze(0),
                              in_=zedge[0:1, 0:HALO + C])

            mark('stage0')
            # ---- stage 0: pe tables ----
            pe_tab = {}
            for t in ("e", "d"):
                psumE = ppA.tile([MAXP, 40], F32, tag="a", name=f"psumE{t}")
                nc.tensor.matmul(psumE[:], lhsT=c_eye17[:], rhs=c_pew1[t][:],
                                 start=True, stop=False)
                nc.tensor.matmul(psumE[:], lhsT=c_ones_row[0:1, 0:MAXP],
                                 rhs=c_peb1[t][:], start=False, stop=True)
                sum17 = work.tile([MAXP, 1], F32, name=f"sum17{t}")
                nc.vector.reduce_sum(sum17[:], psumE[:], axis=AX.X)
                mean17 = work.tile([MAXP, 1], F32, name=f"mean17{t}")
                nc.vector.tensor_scalar(out=mean17[:], in0=sum17[:], scalar1=1.0 / 40,
                                        scalar2=None, op0=OP.mult)
                d17 = work.tile([MAXP, 40], F32, name=f"d17{t}")
                nc.vector.tensor_scalar(out=d17[:], in0=psumE[:], scalar1=mean17[:, 0:1],
                                        scalar2=None, op0=OP.subtract)
                sq17 = work.tile([MAXP, 40], F32, name=f"sq17{t}")
                ssq17 = work.tile([MAXP, 1], F32, name=f"ssq17{t}")
                nc.vector.tensor_tensor(out=sq17[:], in0=d17[:], in1=d17[:], op=OP.mult)
                nc.vector.reduce_sum(ssq17[:], sq17[:], axis=AX.X)
                std17 = work.tile([MAXP, 1], F32, name=f"std17{t}")
                nc.scalar.activation(std17[:], ssq17[:], AF.Sqrt,
                                     bias=c_eps[0:MAXP, 0:1], scale=1.0 / 40)
                rstd17 = work.tile([MAXP, 1], F32, name=f"rstd17{t}")
                nc.vector.reciprocal(rstd17[:], std17[:])
                dn17 = work.tile([MAXP, 40], F32, name=f"dn17{t}")
                nc.vector.scalar_tensor_tensor(out=dn17[:], in0=d17[:], scalar=rstd17[:, 0:1],
                                               in1=c_peg[t][:], op0=OP.mult, op1=OP.mult)
                dn17b = work.tile([MAXP, 40], F32, name=f"dn17b{t}")
                nc.vector.tensor_tensor(out=dn17b[:], in0=dn17[:], in1=c_pebt[t][:], op=OP.add)
                thE = work.tile([MAXP, 41], F32, name=f"thE{t}")
                nc.scalar.activation(thE[:, 0:40], dn17b[:], AF.Tanh)
                nc.gpsimd.memset(thE[:, 40:41], 1.0)
                psumET = ppT.tile([41, MAXP], F32, tag="t", name=f"psumET{t}")
                nc.tensor.transpose(psumET[:], thE[:], c_ident[0:MAXP, 0:MAXP])
                thETe = work.tile([41, MAXP], F32, name=f"thETe{t}")
                nc.vector.tensor_copy(thETe[:], psumET[:])
                psumE2 = ppA.tile([MAXP, 64], F32, tag="a", name=f"psumE2{t}")
                nc.tensor.matmul(psumE2[:], lhsT=thETe[:], rhs=c_pew2e[t][:],
                                 start=True, stop=True)
                ptab = consts.tile([MAXP, 64], F32, name=f"petab{t}")
                nc.vector.tensor_copy(ptab[:], psumE2[:])
                pe_tab[t] = ptab
            psumDT = ppT.tile([64, MAXP], F32, tag="t", name="psumDT")
            nc.tensor.transpose(psumDT[:], pe_tab["d"][:], c_ident[0:MAXP, 0:MAXP])
            peDT = consts.tile([64, MAXP], F32, name="peDT")
            nc.vector.tensor_copy(peDT[:], psumDT[:])
            peDT16 = consts.tile([64, MAXP], DTD, name="peDT16")
            nc.vector.tensor_copy(peDT16[:], peDT[:])

            mark('A')
            # ---- stage A1: x @ w1m; keep d rows, sum-of-squares, mag ----
            for k in range(NK):
                xt4 = work.tile([P, Q * P], F32, name="xt4")
                nc.sync.dma_start(out=xt4[:], in_=xT[:, k * Q * P:(k + 1) * Q * P])
                for q in range(Q):
                    t = k * Q + q
                    psumA = ppA.tile([P, 98], F32, tag="a", name="psumA")
                    nc.tensor.matmul(psumA[:], lhsT=xt4[:, q * P:(q + 1) * P],
                                     rhs=c_w1m[:], start=True, stop=True)
                    nc.vector.tensor_copy(magbuf[:, t:t + 1], psumA[:, 96:97])
                    nc.vector.scalar_tensor_tensor(out=dbuf[:, t, :], in0=psumA[:, 0:96],
                                                   scalar=psumA[:, 97:98], in1=c_bdiff[:],
                                                   op0=OP.subtract, op1=OP.add)
                    dsq = work.tile([P, 96], F32, name="dsq")
                    nc.scalar.activation(dsq[:], dbuf[:, t, :], AF.Square,
                                         accum_out=ssqbuf[:, t:t + 1])
            # ---- stage A2: batched rstd; mag relayout to flat order ----
            stdall = work.tile([P, NT], F32, name="stdall")
            nc.scalar.activation(stdall[:], ssqbuf[:], AF.Sqrt, bias=c_eps[:, 0:1],
                                 scale=1.0 / 96)
            nc.vector.reciprocal(rstdbuf[:], stdall[:])
            for h in range(2):
                hn = NT // 2
                psumM = ppT.tile([hn, P], F32, tag="t", name="psumM")
                nc.tensor.transpose(psumM[:], magbuf[:, h * hn:(h + 1) * hn], c_ident[:])
                magT = work.tile([hn, P], F32, name="magT")
                nc.vector.tensor_copy(magT[:], psumM[:])
                nc.sync.dma_start(
                    out=mag_h[HALO + h * hn * P: HALO + (h + 1) * hn * P]
                    .rearrange("(t p) -> t p", p=P),
                    in_=magT[:])
            # ---- stage A3: normalize + tanh + psi matmul ----
            for k in range(NK):
                th4 = work.tile([P, Q, 97], F32, name="th4")
                psi4 = work.tile([P, Q, 64], F32, name="psi4")
                for q in range(Q):
                    t = k * Q + q
                    dn = work.tile([P, 96], F32, name="dn")
                    nc.vector.scalar_tensor_tensor(out=dn[:], in0=dbuf[:, t, :],
                                                   scalar=rstdbuf[:, t:t + 1],
                                                   in1=c_psig[:], op0=OP.mult, op1=OP.mult)
                    dnb = work.tile([P, 96], F32, name="dnb")
                    nc.gpsimd.tensor_tensor(out=dnb[:], in0=dn[:], in1=c_psibt[:], op=OP.add)
                    nc.scalar.activation(th4[:, q, 0:96], dnb[:], AF.Tanh)
                nc.gpsimd.memset(th4[:, :, 96:97], 1.0)
                for q in range(Q):
                    psumT = ppT.tile([97, P], F32, tag="t", name="psumT")
                    nc.tensor.transpose(psumT[:], th4[:, q, :], c_ident[:])
                    thT97 = work.tile([97, P], F32, name="thT97")
                    nc.vector.tensor_copy(thT97[:], psumT[:])
                    psum2 = ppA.tile([P, 64], F32, tag="a", name="psum2")
                    nc.tensor.matmul(psum2[:], lhsT=thT97[:], rhs=c_psiw2e[:],
                                     start=True, stop=True)
                    nc.vector.tensor_copy(psi4[:, q, :], psum2[:])
                nc.sync.dma_start(
                    out=psi_dram[k * Q * P:(k + 1) * Q * P, :]
                    .rearrange("(q p) h -> p q h", p=P),
                    in_=psi4[:])

            if STOP_AFTER == 'A':
                raise tile.__dict__.get('_never', StopIteration)  # placeholder
            mark('W0')
            # ---- stage W0: window start offsets ----
            seg132 = work.tile([P, C], F32, name="seg132")
            nc.sync.dma_start(out=seg132[:],
                              in_=segh[HALO:HALO + NPAD].rearrange("(p c) -> p c", c=C))
            dummyW = work.tile([P, C], F32, name="dummyW")
            for w in range(NW):
                nc.vector.tensor_scalar(out=dummyW[:], in0=seg132[:],
                                        scalar1=float(w * SEGW), scalar2=None,
                                        op0=OP.is_lt)
                nc.vector.reduce_sum(acc32[:, w:w + 1], dummyW[:], axis=AX.X)
            psumO = ppT.tile([1, NW], F32, tag="t", name="psumO")
            nc.tensor.matmul(psumO[:], lhsT=c_ones_col[:], rhs=acc32[:], start=True, stop=True)
            nc.vector.tensor_copy(orof[:], psumO[:])
            nc.vector.tensor_copy(oroi[:], orof[:])
            # seg relayout to per-tile columns (for stage C)
            for h in range(2):
                hn = NT // 2
                segF = work.tile([hn, P], F32, name="segF")
                nc.sync.dma_start(
                    out=segF[:],
                    in_=segh[HALO + h * hn * P: HALO + (h + 1) * hn * P]
                    .rearrange("(t p) -> t p", p=P))
                psumS = ppT.tile([P, hn], F32, tag="t", name="psumS")
                nc.tensor.transpose(psumS[:], segF[:], c_ident[0:hn, 0:hn])
                nc.vector.tensor_copy(segbufT[:, h * hn:(h + 1) * hn], psumS[:])

            mark('B')
            # ---- stage B: ranks via halo shifts ----
            segt = work.tile([P, C + 2 * HALO], F32, name="segt")
            magt = work.tile([P, C + 2 * HALO], F32, name="magt")
            for (dst, src) in ((segt, segh), (magt, mag_h)):
                nc.sync.dma_start(out=dst[:, HALO:HALO + C],
                                  in_=src[HALO:HALO + NPAD].rearrange("(p c) -> p c", c=C))
                nc.sync.dma_start(out=dst[:, 0:HALO],
                                  in_=src[0:NPAD].rearrange("(p c) -> p c", c=C)[:, 0:HALO])
                nc.sync.dma_start(out=dst[:, C + HALO:C + 2 * HALO],
                                  in_=src[C + HALO:C + HALO + NPAD]
                                  .rearrange("(p c) -> p c", c=C)[:, 0:HALO])
            accR = work.tile([P, C], F32, name="accR")
            nc.vector.memset(accR[:], 0.0)
            W_ = C + 2 * HALO
            for d in range(1, DMAX + 1):
                eq = work.tile([P, W_], F32, name="eq")
                lt = work.tile([P, W_], F32, name="lt")
                pr = work.tile([P, W_], F32, name="pr")
                n_ = W_ - d
                nc.vector.tensor_tensor(out=eq[:, 0:n_], in0=segt[:, 0:n_],
                                        in1=segt[:, d:W_], op=OP.is_equal)
                nc.vector.tensor_tensor(out=lt[:, 0:n_], in0=magt[:, d:W_],
                                        in1=magt[:, 0:n_], op=OP.is_lt)
                nc.vector.tensor_tensor(out=pr[:, 0:n_], in0=eq[:, 0:n_],
                                        in1=lt[:, 0:n_], op=OP.mult)
                nc.vector.tensor_tensor(out=accR[:], in0=accR[:],
                                        in1=pr[:, HALO:HALO + C], op=OP.add)
                nc.vector.tensor_tensor(out=accR[:], in0=accR[:],
                                        in1=eq[:, HALO - d:HALO - d + C], op=OP.add)
                nc.vector.tensor_tensor(out=accR[:], in0=accR[:],
                                        in1=pr[:, HALO - d:HALO - d + C], op=OP.subtract)
            nc.sync.dma_start(out=rankf[:].rearrange("(p c) -> p c", c=C), in_=accR[:])

            mark('tail')
            # ---- y1 tail sentinel rows ----
            ztail = work.tile([P, 66], F32, name="ztail")
            nc.vector.memset(ztail[:], 0.0)
            nc.vector.memset(ztail[:, 64:65], -1.0)
            for k in range(NCHUNK):
                nc.sync.dma_start(out=y1_dram[NPAD + k * P:NPAD + (k + 1) * P, :], in_=ztail[:])

            mark('C')
            # ---- stage C: y1 = psi * pe_enc[rank] ----
            for k in range(NK):
                psi4 = work.tile([P, Q, 64], F32, name="psi4c")
                nc.sync.dma_start(
                    out=psi4[:],
                    in_=psi_dram[k * Q * P:(k + 1) * Q * P, :]
                    .rearrange("(q p) h -> p q h", p=P))
                rrow = work.tile([1, Q * P], F32, name="rrow")
                nc.sync.dma_start(out=rrow[:],
                                  in_=rankf[k * Q * P:(k + 1) * Q * P].unsqueeze(0))
                y14 = work.tile([P, Q, 66], F32, name="y14")
                for q in range(Q):
                    t = k * Q + q
                    rrep = work.tile([MAXP, P], F32, name="rrep")
                    nc.gpsimd.partition_broadcast(rrep[:], rrow[0:1, q * P:(q + 1) * P])
                    oht = work.tile([MAXP, P], F32, name="oht")
                    nc.vector.tensor_tensor(out=oht[:], in0=rrep[:],
                                            in1=c_iota17c[:, 0:1].to_broadcast([MAXP, P]),
                                            op=OP.is_equal)
                    psumP = ppA.tile([P, 64], F32, tag="a", name="psumP")
                    nc.tensor.matmul(psumP[:], lhsT=oht[:], rhs=pe_tab["e"][:],
                                     start=True, stop=True)
                    nc.vector.tensor_tensor(out=y14[:, q, 0:64], in0=psi4[:, q, :],
                                            in1=psumP[:], op=OP.mult)
                    nc.vector.tensor_copy(y14[:, q, 64:65], segbufT[:, t:t + 1])
                nc.gpsimd.memset(y14[:, :, 65:66], 1.0)
                nc.sync.dma_start(
                    out=y1_dram[k * Q * P:(k + 1) * Q * P, :]
                    .rearrange("(q p) h -> p q h", p=P),
                    in_=y14[:])

            mark('WZ')
            # ---- stages W + Z per window ----
            if DYN_GATHER:
                oreg = nc.alloc_register(mybir.EngineType.SP, "oreg")
            else:
                orepall = persist.tile([P, NW], F32, name="orepall")
                nc.gpsimd.partition_broadcast(orepall[:], orof[0:1, :])
            for w in range(NW):
                g = work.tile([P, NCHUNK, 66], F32, name="g")
                if DYN_GATHER:
                    nc.sync.reg_load(oreg, oroi[0:1, w:w + 1])
                    off = nc.snap(oreg, min_val=0, max_val=Y1ROWS - CAP)
                    nc.sync.dma_start(
                        out=g[:],
                        in_=y1_dram[bass.ds(off, CAP), :].rearrange("(c p) d -> p c d", p=P))
                else:
                    idxf = work.tile([P, NCHUNK], F32, name="idxf")
                    nc.vector.tensor_scalar(out=idxf[:], in0=CS("chunkio"),
                                            scalar1=orepall[:, w:w + 1],
                                            scalar2=None, op0=OP.add)
                    idxi = work.tile([P, NCHUNK], I32, name="idxi")
                    nc.vector.tensor_copy(idxi[:], idxf[:])
                    for cch in range(NCHUNK):
                        nc.gpsimd.indirect_dma_start(
                            out=g[:, cch, :], out_offset=None, in_=y1_dram[:, :],
                            in_offset=bass.IndirectOffsetOnAxis(
                                ap=idxi[:, cch:cch + 1], axis=0))
                psumW = ppA.tile([P, 66], F32, tag="a", name="psumW")
                for cch in range(NCHUNK):
                    ohtw = work.tile([P, P], F32, name="ohtw")
                    nc.vector.scalar_tensor_tensor(out=ohtw[:],
                                                   in0=g[:, cch, 64:65].to_broadcast([P, P]),
                                                   scalar=float(-SEGW * w), in1=c_iotaF[:],
                                                   op0=OP.add, op1=OP.is_equal)
                    nc.tensor.matmul(psumW[:], lhsT=ohtw[:], rhs=g[:, cch, :],
                                     start=(cch == 0), stop=(cch == NCHUNK - 1))
                # ---- stage Z ----
                y2s = work.tile([P, 64], F32, name="y2s")
                nc.vector.tensor_copy(y2s[:], psumW[:, 0:64])
                ncol = work.tile([P, 1], F32, name="ncol")
                nc.vector.tensor_copy(ncol[:], psumW[:, 65:66])
                psumZt = ppT.tile([64, P], F32, tag="t", name="psumZt")
                nc.tensor.transpose(psumZt[:], y2s[:], c_ident[:])
                y2T = work.tile([64, P], F32, name="y2T")
                nc.vector.tensor_copy(y2T[:], psumZt[:])
                psumNT = ppT.tile([1, P], F32, tag="t", name="psumNT")
                nc.tensor.transpose(psumNT[:], ncol[:], c_ident[:])
                nrow = work.tile([1, P], F32, name="nrow")
                nc.vector.tensor_copy(nrow[:], psumNT[:])
                nrep = work.tile([MAXP, P], F32, name="nrep")
                nc.gpsimd.partition_broadcast(nrep[:], nrow[0:1, :])
                ohn = work.tile([MAXP, P], F32, name="ohn")
                nc.vector.tensor_tensor(out=ohn[:], in0=nrep[:],
                                        in1=c_iota17c[:, 0:1].to_broadcast([MAXP, P]),
                                        op=OP.is_equal)
                psumH = ppA.tile([P, 72], F32, tag="a", name="psumH")
                nc.tensor.matmul(psumH[:], lhsT=y2T[:], rhs=c_phiw1a[:], start=True, stop=False)
                nc.tensor.matmul(psumH[:], lhsT=ohn[:], rhs=c_phiw1b[:], start=False, stop=True)
                th1 = work.tile([P, 73], F32, name="th1")
                nc.scalar.activation(th1[:, 0:72], psumH[:], AF.Tanh)
                nc.gpsimd.memset(th1[:, 72:73], 1.0)
                psumHT = ppT.tile([73, P], F32, tag="t", name="psumHT")
                nc.tensor.transpose(psumHT[:], th1[:], c_ident[:])
                th1T = work.tile([73, P], F32, name="th1T")
                nc.vector.tensor_copy(th1T[:], psumHT[:])
                psumZ = ppA.tile([P, 64], F32, tag="a", name="psumZ")
                nc.tensor.matmul(psumZ[:], lhsT=th1T[:], rhs=c_phiw2e[:], start=True, stop=True)
                zs = work.tile([P, 64], F32, name="zs")
                nc.vector.tensor_copy(zs[:], psumZ[:])
                psumZT2 = ppT.tile([64, P], F32, tag="t", name="psumZT2")
                nc.tensor.transpose(psumZT2[:], zs[:], c_ident[:])
                nc.vector.tensor_copy(zTbuf[0:64, w * P:(w + 1) * P], psumZT2[:])
                nc.vector.tensor_copy(zTbuf16[:, w * P:(w + 1) * P], psumZT2[:])
                # size head + argmax
                psumL1 = ppA.tile([P, 40], F32, tag="a", name="psumL1")
                nc.tensor.matmul(psumL1[:], lhsT=zTbuf[:, w * P:(w + 1) * P],
                                 rhs=c_sizew1e[:], start=True, stop=True)
                th2 = work.tile([P, 41], F32, name="th2")
                nc.scalar.activation(th2[:, 0:40], psumL1[:], AF.Tanh)
                nc.gpsimd.memset(th2[:, 40:41], 1.0)
                psumT2 = ppT.tile([41, P], F32, tag="t", name="psumT2")
                nc.tensor.transpose(psumT2[:], th2[:], c_ident[:])
                th2T = work.tile([41, P], F32, name="th2T")
                nc.vector.tensor_copy(th2T[:], psumT2[:])
                psumLg = ppA.tile([P, MAXP], F32, tag="a", name="psumLg")
                nc.tensor.matmul(psumLg[:], lhsT=th2T[:], rhs=c_sizew2e[:], start=True, stop=True)
                mx = work.tile([P, 1], F32, name="mx")
                nc.vector.reduce_max(mx[:], psumLg[:], axis=AX.X)
                eqm = work.tile([P, MAXP], F32, name="eqm")
                nc.vector.tensor_scalar(out=eqm[:], in0=psumLg[:], scalar1=mx[:, 0:1],
                                        scalar2=None, op0=OP.is_equal)
                vv = work.tile([P, MAXP], F32, name="vv")
                nc.vector.tensor_tensor(out=vv[:], in0=eqm[:], in1=c_rev17[:], op=OP.mult)
                mv = work.tile([P, 1], F32, name="mv")
                nc.vector.reduce_max(mv[:], vv[:], axis=AX.X)
                ndec = work.tile([P, 1], F32, name="ndec")
                nc.vector.tensor_scalar(out=ndec[:], in0=mv[:], scalar1=-1.0, scalar2=float(MAXP),
                                        op0=OP.mult, op1=OP.add)
                nc.vector.tensor_scalar(out=mask_buf[:, w * MAXP:(w + 1) * MAXP],
                                        in0=c_iota17r[:], scalar1=ndec[:, 0:1],
                                        scalar2=None, op0=OP.is_lt)
                mrow = work.tile([P, MAXP], F32, name="mrow")
                nc.vector.tensor_copy(mrow[:], mask_buf[:, w * MAXP:(w + 1) * MAXP])
                nc.sync.dma_start(out=out_mask[w * P:(w + 1) * P, :], in_=mrow[:])

            mark('D')
            # ---- stage D: decoder (bf16 matmuls) ----
            out_v = out_xr[:, :].rearrange("(b p) d -> b p d", p=MAXP)
            groups = [(0, 4), (4, 4), (8, 4), (12, 4), (16, 1)]
            for s in range(NW):
                for (p0, G) in groups:
                    zpT4 = work.tile([64, G, P], DTD, name="zpT4")
                    nc.vector.tensor_tensor(
                        out=zpT4[:],
                        in0=zTbuf16[:, s * P:(s + 1) * P].unsqueeze(1).to_broadcast([64, G, P]),
                        in1=peDT16[:, p0:p0 + G].unsqueeze(2).to_broadcast([64, G, P]),
                        op=OP.mult)
                    psumD2 = ppA.tile([P, G * P], F32, tag="a", name="psumD2")
                    psumD14 = ppT.tile([96, G, P], F32, tag="t", name="psumD14")
                    thD4 = work.tile([97, G, P], DTD, name="thD4")
                    nc.gpsimd.memset(thD4[96:97, :, :], 1.0)
                    for pi in range(G):
                        nc.tensor.matmul(psumD14[:, pi, :], lhsT=c16_decw1[:],
                                         rhs=zpT4[:, pi, :], start=True, stop=True)
                    nc.scalar.activation(thD4[0:96, :, :], psumD14[:], AF.Tanh,
                                         bias=c_decb1c[:, 0:1])
                    for pi in range(G):
                        nc.tensor.matmul(psumD2[:, pi * P:(pi + 1) * P], lhsT=thD4[:, pi, :],
                                         rhs=c16_decw2e[:], start=True, stop=True)
                    xrs = work.tile([P, G, P], F32, name="xrs")
                    nc.vector.tensor_tensor(
                        out=xrs[:],
                        in0=psumD2[:].rearrange("p (g d) -> p g d", g=G),
                        in1=mask_buf[:, s * MAXP + p0:s * MAXP + p0 + G]
                        .unsqueeze(2).to_broadcast([P, G, P]),
                        op=OP.mult)
                    nc.sync.dma_start(out=out_v[s * P:(s + 1) * P, p0:p0 + G, :],
                                      in_=xrs[:])
    mark('end')
    nc.compile()
    return nc


def _prep_host(inputs):
    """Shard inputs + build constant tensors. Returns in_maps for 8 cores."""
    gi = lambda k: np.asarray(inputs[k], np.float32)
    x = gi("x")
    batch = np.asarray(inputs["batch"]).astype(np.int64)

    counts = np.bincount(batch, minlength=B)
    assert counts.max() <= 16, f"segment size {counts.max()} > 16 unsupported"

    cv = {}
    psi_w1 = gi("psi_w1")
    rank_w = gi("rank_w")
    cv["w1m"] = np.concatenate(
        [psi_w1, rank_w, psi_w1.mean(axis=1, keepdims=True)], axis=1)
    psi_b1 = gi("psi_b1")
    cv["bdiff"] = np.broadcast_to(psi_b1 - psi_b1.mean(), (P, 96))
    cv["psi_g"] = np.broadcast_to(gi("psi_g"), (P, 96))
    cv["psi_bt"] = np.broadcast_to(gi("psi_bt"), (P, 96))
    cv["psi_w2e"] = np.concatenate([gi("psi_w2"), gi("psi_b2")[None, :]], axis=0)
    cv["ident"] = np.eye(P, dtype=np.float32)
    cv["eye17"] = np.eye(MAXP, dtype=np.float32)
    cv["ones_row"] = np.ones((1, P), np.float32)
    cv["ones_col"] = np.ones((P, 1), np.float32)
    for t in ("e", "d"):
        cv[f"pe_{t}_w1"] = gi(f"pe_{t}_w1")
        cv[f"pe_{t}_b1"] = gi(f"pe_{t}_b1")[None, :]
        cv[f"pe_{t}_g"] = np.broadcast_to(gi(f"pe_{t}_g"), (MAXP, 40))
        cv[f"pe_{t}_bt"] = np.broadcast_to(gi(f"pe_{t}_bt"), (MAXP, 40))
        cv[f"pe_{t}_w2e"] = np.concatenate(
            [gi(f"pe_{t}_w2"), gi(f"pe_{t}_b2")[None, :]], axis=0)
    phi_w1 = gi("phi_w1")
    cv["phi_w1a"] = phi_w1[:64]
    cv["phi_w1b"] = phi_w1[64:64 + MAXP] + gi("phi_b1")[None, :]
    cv["phi_w2e"] = np.concatenate([gi("phi_w2"), gi("phi_b2")[None, :]], axis=0)
    cv["size_w1e"] = np.concatenate([gi("size_w1"), gi("size_b1")[None, :]], axis=0)
    cv["size_w2e"] = np.concatenate([gi("size_w2"), gi("size_b2")[None, :]], axis=0)
    cv["dec_b1c"] = gi("dec_b1")[:, None]
    cv["iota17c"] = np.arange(MAXP, dtype=np.float32)[:, None]
    cv["iotaF"] = np.broadcast_to(np.arange(P, dtype=np.float32), (P, P))
    cv["chunkio"] = (np.arange(P, dtype=np.float32)[:, None]
                     + P * np.arange(NCHUNK, dtype=np.float32)[None, :])
    cv["rev17"] = np.broadcast_to(
        float(MAXP) - np.arange(MAXP, dtype=np.float32), (P, MAXP))
    cv["iota17r"] = np.broadcast_to(np.arange(MAXP, dtype=np.float32), (P, MAXP))

    cpack = np.zeros((P, CPCOLS), np.float32)
    for name, r, c in _CONST_SPECS:
        v = np.asarray(cv[name], np.float32)
        assert v.shape == (r, c), f"{name}: {v.shape} != {(r, c)}"
        off = _CONST_OFF[name][0]
        cpack[:r, off:off + c] = v

    cv16 = {
        "dec_w1": gi("dec_w1"),
        "dec_w2e": np.concatenate([gi("dec_w2"), gi("dec_b2")[None, :]], axis=0),
    }
    dtd = ml_dtypes.bfloat16 if BF16_D else np.float32
    cpack16 = np.zeros((P, CP16COLS), dtd)
    for name, r, c in _CONST16_SPECS:
        v = np.asarray(cv16[name], np.float32).astype(dtd)
        assert v.shape == (r, c), f"{name}: {v.shape} != {(r, c)}"
        off = _CONST16_OFF[name][0]
        cpack16[:r, off:off + c] = v

    base = {"cpack": cpack, "cpack16": cpack16}

    bounds = np.searchsorted(batch, np.arange(0, B + 1, BLOC))
    in_maps = []
    for c in range(NCORES):
        s, e = int(bounds[c]), int(bounds[c + 1])
        nloc = e - s
        assert nloc <= NPAD, f"core {c} has {nloc} > {NPAD} elements"
        wb = np.searchsorted(batch[s:e], c * BLOC + np.arange(0, BLOC + 1, SEGW))
        assert np.diff(wb).max() <= CAP, f"window overflow on core {c}"
        xTc = np.zeros((P, NPAD), np.float32)
        xTc[:, :nloc] = x[s:e].T
        seghc = np.full(SEGH_LEN, BIGSEG, np.float32)
        seghc[HALO:HALO + nloc] = (batch[s:e] - c * BLOC).astype(np.float32)
        m = dict(base)
        m["xT"] = xTc
        m["segh"] = seghc
        in_maps.append(m)
    return in_maps


def _run(inputs, trace=False):
    if "nc" not in _CACHED:
        _CACHED["nc"] = _build_bass()
    nc = _CACHED["nc"]
    in_maps = _prep_host(inputs)
    res = run_bass_kernel_spmd(nc, in_maps, list(range(NCORES)), trace=trace)
    outs = res.results
    xr = np.concatenate([outs[c]["out_xr"] for c in range(NCORES)], axis=0)
    maskf = np.concatenate([outs[c]["out_mask"] for c in range(NCORES)], axis=0)
    mask = maskf.reshape(-1) > 0.5
    batchr = np.repeat(np.arange(B, dtype=np.int32), MAXP)
    return (xr, batchr, mask), res.exec_time_ns


def kernel(**inputs):
    out, _ = _run(inputs, trace=False)
    return out


def kernel_traced(**inputs):
    return _run(inputs, trace=True)


# revision 52
# speedup vs baseline: 1.0203x; 1.0203x over previous
"""Trainium2 Bass kernel for nn_AutoEncoder_75282186764670.

Set autoencoder over ragged segments (B=32768 sets, N=131072 elements,
DIM=128, HID=64, MAXP=17).  Data-parallel over sets: each of the 8 cores
owns 4096 consecutive segment ids and their contiguous x rows.

Per-core pipeline:
  A1 per 128-element tile (batched 4/DMA): psumA = x @ [psi_w1|rank_w|rowmean]
     -> centered d rows (kept in SBUF), sum-of-squares, mag column
  A2 batched sqrt/reciprocal for all LayerNorm rstd; mag relayout to flat
     element order via PE transpose
  A3 normalize+tanh -> psi = th @ [psi_w2;psi_b2] -> psi_dram (batched)
  B  ranks: halo layout [128, 132+2*16]; rank_i = #{j in seg: mag_j < mag_i}
     + #{j<i in seg: mag_j == mag_i} via shifted compares (segments <= 16)
  C  y1 = psi * pe_enc[rank] via transposed one-hot matmul against the
     pe table; y1 rows + seg id + ones -> y1_dram [Npad+CAP, 66] (batched)
  W  per 128-segment window: ONE dynamic-offset DMA pulls its <=768
     element rows; one-hot(seg - 128w) matmul accumulate -> y2, counts n
  Z  z = phi(concat(y2, onehot(n))), logits = size(z), n_dec = argmax,
     mask rows -> DRAM, zT kept in SBUF
  D  decoder (bf16 matmuls): hT = tanh(w1.T @ (zT*pe_dec_T[:,p]) + b1),
     xr = [hT;1].T @ [w2;b2], masked by (p < n_dec), rows (b*17+p).

Notes:
 - rank_b is dropped: a constant shift never changes within-segment order.
 - psi_b1 enters LN only as (b1 - mean(b1)) added after mean-subtraction;
   the row-mean of x@w1 comes from an extra matmul column (w1 @ 1/96).
 - one_hot(n) always sums to 1 here (n <= 16 asserted), so phi_b1 is folded
   into the one-hot weight table.
 - harness pre-zeroes ExternalOutput buffers.
"""

import sys

import numpy as np

for _p in ("/opt/trn_rl_repo",):
    if _p not in sys.path:
        sys.path.insert(0, _p)

import ml_dtypes

import concourse.bacc as bacc
import concourse.bass as bass
import concourse.mybir as mybir
import concourse.tile as tile
from concourse.bass_utils import run_bass_kernel_spmd

F32 = mybir.dt.float32
BF16 = mybir.dt.bfloat16
I32 = mybir.dt.int32
AF = mybir.ActivationFunctionType
OP = mybir.AluOpType
AX = mybir.AxisListType

P = 128
DIM = 128
HID = 64
MAXP = 17
NCORES = 8
B = 32768
N = 131072
BLOC = B // NCORES          # segments per core
NPAD = 16896                # element capacity per core (132 * 128)
C = NPAD // P               # 132 columns in halo layout
NT = NPAD // P              # 132 element tiles
Q = 4                       # element tiles per DMA batch
NK = NT // Q                # 33
HALO = 16
DMAX = 15                   # max within-segment distance (seg size <= 16)
SEGW = 128                  # segments per window
NW = BLOC // SEGW           # 32 windows
CAP = 768                   # element capacity per window
NCHUNK = CAP // P           # 6
Y1ROWS = NPAD + CAP
SEGH_LEN = HALO + NPAD + HALO + C   # extra C so right-halo rearrange stays in bounds
BIGSEG = 1.0e6
EPS = 1e-5

BF16_D = True
DYN_GATHER = True
STOP_AFTER = None  # 'A'|'B'|'C'|'WZ' truncates the program for HW bisection
_CACHED = {}
_STAGE_MARKS = []   # (stage_name, first_inst_id) in build order

# All replicated f32 constants packed into one [128, X] tensor (one DMA).
_CONST_SPECS = [
    ("w1m", 128, 98), ("bdiff", 128, 96), ("psi_g", 128, 96), ("psi_bt", 128, 96),
    ("psi_w2e", 97, 64),
    ("ident", 128, 128), ("eye17", 17, 17), ("ones_row", 1, 128), ("ones_col", 128, 1),
    ("pe_e_w1", 17, 40), ("pe_e_b1", 1, 40), ("pe_e_g", 17, 40), ("pe_e_bt", 17, 40),
    ("pe_e_w2e", 41, 64),
    ("pe_d_w1", 17, 40), ("pe_d_b1", 1, 40), ("pe_d_g", 17, 40), ("pe_d_bt", 17, 40),
    ("pe_d_w2e", 41, 64),
    ("phi_w1a", 64, 72), ("phi_w1b", 17, 72), ("phi_w2e", 73, 64),
    ("size_w1e", 65, 40), ("size_w2e", 41, 17),
    ("dec_b1c", 96, 1),
    ("iota17c", 17, 1), ("iotaF", 128, 128), ("chunkio", 128, 6),
    ("rev17", 128, 17), ("iota17r", 128, 17),
]
_CONST_OFF = {}
_off = 0
for _n, _r, _c in _CONST_SPECS:
    _CONST_OFF[_n] = (_off, _r, _c)
    _off += _c
CPCOLS = _off

# bf16 decoder weights packed the same way.
_CONST16_SPECS = [("dec_w1", 64, 96), ("dec_w2e", 97, 128)]
_CONST16_OFF = {}
_off = 0
for _n, _r, _c in _CONST16_SPECS:
    _CONST16_OFF[_n] = (_off, _r, _c)
    _off += _c
CP16COLS = _off


def _build_bass():
    nc = bacc.Bacc("TRN2", target_bir_lowering=False, debug=False)
    _STAGE_MARKS.clear()

    def mark(name):
        _STAGE_MARKS.append((name, nc.next_id()))

    # per-core (sharded) inputs
    xT = nc.declare_dram_parameter("xT", [P, NPAD], F32, isOutput=False)
    segh = nc.declare_dram_parameter("segh", [SEGH_LEN], F32, isOutput=False)
    cpack = nc.declare_dram_parameter("cpack", [P, CPCOLS], F32, isOutput=False)
    DTD = BF16 if BF16_D else F32
    cpack16 = nc.declare_dram_parameter("cpack16", [P, CP16COLS], DTD, isOutput=False)

    out_xr = nc.declare_dram_parameter("out_xr", [BLOC * MAXP, DIM], F32, isOutput=True)
    out_mask = nc.declare_dram_parameter("out_mask", [BLOC, MAXP], F32, isOutput=True)

    # internal DRAM scratch
    mag_h = nc.dram_tensor("mag_h", [SEGH_LEN], F32)
    rankf = nc.dram_tensor("rankf", [NPAD], F32)
    psi_dram = nc.dram_tensor("psi_dram", [NPAD, 64], F32)
    y1_dram = nc.dram_tensor("y1_dram", [Y1ROWS, 66], F32)

    with tile.TileContext(nc) as tc:
        import contextlib

        with contextlib.ExitStack() as ctx:
            consts = ctx.enter_context(tc.tile_pool(name="consts", bufs=1))
            persist = ctx.enter_context(tc.tile_pool(name="persist", bufs=1))
            work = ctx.enter_context(tc.tile_pool(name="work", bufs=4))
            ppA = ctx.enter_context(tc.tile_pool(name="ppA", bufs=4, space="PSUM"))
            ppT = ctx.enter_context(tc.tile_pool(name="ppT", bufs=4, space="PSUM"))

            # ---- constants: one DMA each ----
            cbuf = consts.tile([P, CPCOLS], F32, name="cbuf")
            nc.sync.dma_start(out=cbuf[:], in_=cpack[:, :])
            cbuf16 = consts.tile([P, CP16COLS], DTD, name="cbuf16")
            nc.sync.dma_start(out=cbuf16[:], in_=cpack16[:, :])

            def CS(name):
                off, r, c = _CONST_OFF[name]
                return cbuf[0:r, off:off + c]

            def CS16(name):
                off, r, c = _CONST16_OFF[name]
                return cbuf16[0:r, off:off + c]

            c_w1m = CS("w1m")
            c_bdiff = CS("bdiff")
            c_psig = CS("psi_g")
            c_psibt = CS("psi_bt")
            c_psiw2e = CS("psi_w2e")
            c_ident = CS("ident")
            c_eye17 = CS("eye17")
            c_ones_row = CS("ones_row")
            c_ones_col = CS("ones_col")
            c_pew1 = {t: CS(f"pe_{t}_w1") for t in ("e", "d")}
            c_peb1 = {t: CS(f"pe_{t}_b1") for t in ("e", "d")}
            c_peg = {t: CS(f"pe_{t}_g") for t in ("e", "d")}
            c_pebt = {t: CS(f"pe_{t}_bt") for t in ("e", "d")}
            c_pew2e = {t: CS(f"pe_{t}_w2e") for t in ("e", "d")}
            c_phiw1a = CS("phi_w1a")
            c_phiw1b = CS("phi_w1b")
            c_phiw2e = CS("phi_w2e")
            c_sizew1e = CS("size_w1e")
            c_sizew2e = CS("size_w2e")
            c_decb1c = CS("dec_b1c")
            c_iota17c = CS("iota17c")
            c_iotaF = CS("iotaF")
            c_rev17 = CS("rev17")
            c_iota17r = CS("iota17r")
            c16_decw1 = CS16("dec_w1")
            c16_decw2e = CS16("dec_w2e")

            # persistent SBUF state
            zTbuf = persist.tile([65, BLOC], F32, name="zTbuf")      # row 64 = ones
            zTbuf16 = persist.tile([64, BLOC], DTD, name="zTbuf16")
            mask_buf = persist.tile([P, NW * MAXP], F32, name="mask_buf")
            dbuf = persist.tile([P, NT, 96], F32, name="dbuf")
            ssqbuf = persist.tile([P, NT], F32, name="ssqbuf")
            rstdbuf = persist.tile([P, NT], F32, name="rstdbuf")
            magbuf = persist.tile([P, NT], F32, name="magbuf")
            segbufT = persist.tile([P, NT], F32, name="segbufT")
            acc32 = persist.tile([P, NW], F32, name="acc32")
            oroi = persist.tile([1, NW], I32, name="oroi")
            orof = persist.tile([1, NW], F32, name="orof")
            nc.gpsimd.memset(zTbuf[64:65, :], 1.0)
            dsqd = persist.tile([P, 96], F32, name="dsqd")
            c_eps = persist.tile([P, 1], F32, name="c_eps")
            nc.gpsimd.memset(c_eps[:], EPS)

            # zero mag_h halo edges (never written by stage A2, read by stage B)
            zedge = persist.tile([1, C + 2 * HALO], F32, name="zedge")
            nc.vector.memset(zedge[:], 0.0)
            nc.sync.dma_start(out=mag_h[0:HALO].unsqueeze(0), in_=zedge[0:1, 0:HALO])
            nc.sync.dma_start(out=mag_h[HALO + NPAD:SEGH_LEN].unsqueeze(0),
                              in_=zedge[0:1, 0:HALO + C])

            mark('stage0')
            # ---- stage 0: pe tables ----
            pe_tab = {}
            for t in ("e", "d"):
                psumE = ppA.tile([MAXP, 40], F32, tag="a", name=f"psumE{t}")
                nc.tensor.matmul(psumE[:], lhsT=c_eye17[:], rhs=c_pew1[t][:],
                                 start=True, stop=False)
                nc.tensor.matmul(psumE[:], lhsT=c_ones_row[0:1, 0:MAXP],
                                 rhs=c_peb1[t][:], start=False, stop=True)
                sum17 = work.tile([MAXP, 1], F32, name=f"sum17{t}")
                nc.vector.reduce_sum(sum17[:], psumE[:], axis=AX.X)
                mean17 = work.tile([MAXP, 1], F32, name=f"mean17{t}")
                nc.vector.tensor_scalar(out=mean17[:], in0=sum17[:], scalar1=1.0 / 40,
                                        scalar2=None, op0=OP.mult)
                d17 = work.tile([MAXP, 40], F32, name=f"d17{t}")
                nc.vector.tensor_scalar(out=d17[:], in0=psumE[:], scalar1=mean17[:, 0:1],
                                        scalar2=None, op0=OP.subtract)
                sq17 = work.tile([MAXP, 40], F32, name=f"sq17{t}")
                ssq17 = work.tile([MAXP, 1], F32, name=f"ssq17{t}")
                nc.vector.tensor_tensor(out=sq17[:], in0=d17[:], in1=d17[:], op=OP.mult)
                nc.vector.reduce_sum(ssq17[:], sq17[:], axis=AX.X)
                std17 = work.tile([MAXP, 1], F32, name=f"std17{t}")
                nc.scalar.activation(std17[:], ssq17[:], AF.Sqrt,
                                     bias=c_eps[0:MAXP, 0:1], scale=1.0 / 40)
                rstd17 = work.tile([MAXP, 1], F32, name=f"rstd17{t}")
                nc.vector.reciprocal(rstd17[:], std17[:])
                dn17 = work.tile([MAXP, 40], F32, name=f"dn17{t}")
                nc.vector.scalar_tensor_tensor(out=dn17[:], in0=d17[:], scalar=rstd17[:, 0:1],
                                               in1=c_peg[t][:], op0=OP.mult, op1=OP.mult)
                dn17b = work.tile([MAXP, 40], F32, name=f"dn17b{t}")
                nc.vector.tensor_tensor(out=dn17b[:], in0=dn17[:], in1=c_pebt[t][:], op=OP.add)
                thE = work.tile([MAXP, 41], F32, name=f"thE{t}")
                nc.scalar.activation(thE[:, 0:40], dn17b[:], AF.Tanh)
                nc.gpsimd.memset(thE[:, 40:41], 1.0)
                psumET = ppT.tile([41, MAXP], F32, tag="t", name=f"psumET{t}")
                nc.tensor.transpose(psumET[:], thE[:], c_ident[0:MAXP, 0:MAXP])
                thETe = work.tile([41, MAXP], F32, name=f"thETe{t}")
                nc.vector.tensor_copy(thETe[:], psumET[:])
                psumE2 = ppA.tile([MAXP, 64], F32, tag="a", name=f"psumE2{t}")
                nc.tensor.matmul(psumE2[:], lhsT=thETe[:], rhs=c_pew2e[t][:],
                                 start=True, stop=True)
                ptab = consts.tile([MAXP, 64], F32, name=f"petab{t}")
                nc.vector.tensor_copy(ptab[:], psumE2[:])
                pe_tab[t] = ptab
            psumDT = ppT.tile([64, MAXP], F32, tag="t", name="psumDT")
            nc.tensor.transpose(psumDT[:], pe_tab["d"][:], c_ident[0:MAXP, 0:MAXP])
            peDT = consts.tile([64, MAXP], F32, name="peDT")
            nc.vector.tensor_copy(peDT[:], psumDT[:])
            peDT16 = consts.tile([64, MAXP], DTD, name="peDT16")
            nc.vector.tensor_copy(peDT16[:], peDT[:])

            mark('A')
            # ---- stage A1: x @ w1m; keep d rows, sum-of-squares, mag ----
            for k in range(NK):
                xt4 = work.tile([P, Q * P], F32, name="xt4")
                nc.sync.dma_start(out=xt4[:], in_=xT[:, k * Q * P:(k + 1) * Q * P])
                for q in range(Q):
                    t = k * Q + q
                    psumA = ppA.tile([P, 98], F32, tag="a", name="psumA")
                    nc.tensor.matmul(psumA[:], lhsT=xt4[:, q * P:(q + 1) * P],
                                     rhs=c_w1m[:], start=True, stop=True)
                    nc.vector.tensor_copy(magbuf[:, t:t + 1], psumA[:, 96:97])
                    nc.vector.scalar_tensor_tensor(out=dbuf[:, t, :], in0=psumA[:, 0:96],
                                                   scalar=psumA[:, 97:98], in1=c_bdiff[:],
                                                   op0=OP.subtract, op1=OP.add)
                    nc.scalar.activation(dsqd[:], dbuf[:, t, :], AF.Square,
                                         accum_out=ssqbuf[:, t:t + 1])
            # ---- stage A2: batched rstd; mag relayout to flat order ----
            stdall = work.tile([P, NT], F32, name="stdall")
            nc.scalar.activation(stdall[:], ssqbuf[:], AF.Sqrt, bias=c_eps[:, 0:1],
                                 scale=1.0 / 96)
            nc.vector.reciprocal(rstdbuf[:], stdall[:])
            for h in range(2):
                hn = NT // 2
                psumM = ppT.tile([hn, P], F32, tag="t", name="psumM")
                nc.tensor.transpose(psumM[:], magbuf[:, h * hn:(h + 1) * hn], c_ident[:])
                magT = work.tile([hn, P], F32, name="magT")
                nc.vector.tensor_copy(magT[:], psumM[:])
                nc.sync.dma_start(
                    out=mag_h[HALO + h * hn * P: HALO + (h + 1) * hn * P]
                    .rearrange("(t p) -> t p", p=P),
                    in_=magT[:])
            # ---- stage A3: normalize + tanh + psi matmul ----
            for k in range(NK):
                th4 = work.tile([P, Q, 97], F32, name="th4")
                psi4 = work.tile([P, Q, 64], F32, name="psi4")
                for q in range(Q):
                    t = k * Q + q
                    dn = work.tile([P, 96], F32, name="dn")
                    nc.vector.scalar_tensor_tensor(out=dn[:], in0=dbuf[:, t, :],
                                                   scalar=rstdbuf[:, t:t + 1],
                                                   in1=c_psig[:], op0=OP.mult, op1=OP.mult)
                    dnb = work.tile([P, 96], F32, name="dnb")
                    nc.gpsimd.tensor_tensor(out=dnb[:], in0=dn[:], in1=c_psibt[:], op=OP.add)
                    nc.scalar.activation(th4[:, q, 0:96], dnb[:], AF.Tanh)
                nc.gpsimd.memset(th4[:, :, 96:97], 1.0)
                for q in range(Q):
                    psumT = ppT.tile([97, P], F32, tag="t", name="psumT")
                    nc.tensor.transpose(psumT[:], th4[:, q, :], c_ident[:])
                    thT97 = work.tile([97, P], F32, name="thT97")
                    nc.vector.tensor_copy(thT97[:], psumT[:])
                    psum2 = ppA.tile([P, 64], F32, tag="a", name="psum2")
                    nc.tensor.matmul(psum2[:], lhsT=thT97[:], rhs=c_psiw2e[:],
                                     start=True, stop=True)
                    nc.vector.tensor_copy(psi4[:, q, :], psum2[:])
                nc.sync.dma_start(
                    out=psi_dram[k * Q * P:(k + 1) * Q * P, :]
                    .rearrange("(q p) h -> p q h", p=P),
                    in_=psi4[:])

            if STOP_AFTER == 'A':
                raise tile.__dict__.get('_never', StopIteration)  # placeholder
            mark('W0')
            # ---- stage W0: window start offsets ----
            seg132 = work.tile([P, C], F32, name="seg132")
            nc.sync.dma_start(out=seg132[:],
                              in_=segh[HALO:HALO + NPAD].rearrange("(p c) -> p c", c=C))
            dummyW = work.tile([P, C], F32, name="dummyW")
            for w in range(NW):
                nc.vector.tensor_scalar(out=dummyW[:], in0=seg132[:],
                                        scalar1=float(w * SEGW), scalar2=None,
                                        op0=OP.is_lt)
                nc.vector.reduce_sum(acc32[:, w:w + 1], dummyW[:], axis=AX.X)
            psumO = ppT.tile([1, NW], F32, tag="t", name="psumO")
            nc.tensor.matmul(psumO[:], lhsT=c_ones_col[:], rhs=acc32[:], start=True, stop=True)
            nc.vector.tensor_copy(orof[:], psumO[:])
            nc.vector.tensor_copy(oroi[:], orof[:])
            # seg relayout to per-tile columns (for stage C)
            for h in range(2):
                hn = NT // 2
                segF = work.tile([hn, P], F32, name="segF")
                nc.sync.dma_start(
                    out=segF[:],
                    in_=segh[HALO + h * hn * P: HALO + (h + 1) * hn * P]
                    .rearrange("(t p) -> t p", p=P))
                psumS = ppT.tile([P, hn], F32, tag="t", name="psumS")
                nc.tensor.transpose(psumS[:], segF[:], c_ident[0:hn, 0:hn])
                nc.vector.tensor_copy(segbufT[:, h * hn:(h + 1) * hn], psumS[:])

            mark('B')
            # ---- stage B: ranks via halo shifts ----
            segt = work.tile([P, C + 2 * HALO], F32, name="segt")
            magt = work.tile([P, C + 2 * HALO], F32, name="magt")
            for (dst, src) in ((segt, segh), (magt, mag_h)):
                nc.sync.dma_start(out=dst[:, HALO:HALO + C],
                                  in_=src[HALO:HALO + NPAD].rearrange("(p c) -> p c", c=C))
                nc.sync.dma_start(out=dst[:, 0:HALO],
                                  in_=src[0:NPAD].rearrange("(p c) -> p c", c=C)[:, 0:HALO])
                nc.sync.dma_start(out=dst[:, C + HALO:C + 2 * HALO],
                                  in_=src[C + HALO:C + HALO + NPAD]
                                  .rearrange("(p c) -> p c", c=C)[:, 0:HALO])
            accR = work.tile([P, C], F32, name="accR")
            nc.vector.memset(accR[:], 0.0)
            W_ = C + 2 * HALO
            for d in range(1, DMAX + 1):
                eq = work.tile([P, W_], F32, name="eq")
                lt = work.tile([P, W_], F32, name="lt")
                pr = work.tile([P, W_], F32, name="pr")
                n_ = W_ - d
                nc.vector.tensor_tensor(out=eq[:, 0:n_], in0=segt[:, 0:n_],
                                        in1=segt[:, d:W_], op=OP.is_equal)
                nc.vector.tensor_tensor(out=lt[:, 0:n_], in0=magt[:, d:W_],
                                        in1=magt[:, 0:n_], op=OP.is_lt)
                nc.vector.tensor_tensor(out=pr[:, 0:n_], in0=eq[:, 0:n_],
                                        in1=lt[:, 0:n_], op=OP.mult)
                nc.vector.tensor_tensor(out=accR[:], in0=accR[:],
                                        in1=pr[:, HALO:HALO + C], op=OP.add)
                nc.vector.tensor_tensor(out=accR[:], in0=accR[:],
                                        in1=eq[:, HALO - d:HALO - d + C], op=OP.add)
                nc.vector.tensor_tensor(out=accR[:], in0=accR[:],
                                        in1=pr[:, HALO - d:HALO - d + C], op=OP.subtract)
            nc.sync.dma_start(out=rankf[:].rearrange("(p c) -> p c", c=C), in_=accR[:])

            mark('tail')
            # ---- y1 tail sentinel rows ----
            ztail = work.tile([P, 66], F32, name="ztail")
            nc.vector.memset(ztail[:], 0.0)
            nc.vector.memset(ztail[:, 64:65], -1.0)
            for k in range(NCHUNK):
                nc.sync.dma_start(out=y1_dram[NPAD + k * P:NPAD + (k + 1) * P, :], in_=ztail[:])

            mark('C')
            # ---- stage C: y1 = psi * pe_enc[rank] ----
            for k in range(NK):
                psi4 = work.tile([P, Q, 64], F32, name="psi4c")
                nc.sync.dma_start(
                    out=psi4[:],
                    in_=psi_dram[k * Q * P:(k + 1) * Q * P, :]
                    .rearrange("(q p) h -> p q h", p=P))
                rrow = work.tile([1, Q * P], F32, name="rrow")
                nc.sync.dma_start(out=rrow[:],
                                  in_=rankf[k * Q * P:(k + 1) * Q * P].unsqueeze(0))
                y14 = work.tile([P, Q, 66], F32, name="y14")
                for q in range(Q):
                    t = k * Q + q
                    rrep = work.tile([MAXP, P], F32, name="rrep")
                    nc.gpsimd.partition_broadcast(rrep[:], rrow[0:1, q * P:(q + 1) * P])
                    oht = work.tile([MAXP, P], F32, name="oht")
                    nc.vector.tensor_tensor(out=oht[:], in0=rrep[:],
                                            in1=c_iota17c[:, 0:1].to_broadcast([MAXP, P]),
                                            op=OP.is_equal)
                    psumP = ppA.tile([P, 64], F32, tag="a", name="psumP")
                    nc.tensor.matmul(psumP[:], lhsT=oht[:], rhs=pe_tab["e"][:],
                                     start=True, stop=True)
                    nc.vector.tensor_tensor(out=y14[:, q, 0:64], in0=psi4[:, q, :],
                                            in1=psumP[:], op=OP.mult)
                    nc.vector.tensor_copy(y14[:, q, 64:65], segbufT[:, t:t + 1])
                nc.gpsimd.memset(y14[:, :, 65:66], 1.0)
                nc.sync.dma_start(
                    out=y1_dram[k * Q * P:(k + 1) * Q * P, :]
                    .rearrange("(q p) h -> p q h", p=P),
                    in_=y14[:])

            mark('WZ')
            # ---- stages W + Z per window ----
            if DYN_GATHER:
                oreg = nc.alloc_register(mybir.EngineType.SP, "oreg")
            else:
                orepall = persist.tile([P, NW], F32, name="orepall")
                nc.gpsimd.partition_broadcast(orepall[:], orof[0:1, :])
            for w in range(NW):
                g = work.tile([P, NCHUNK, 66], F32, name="g")
                if DYN_GATHER:
                    nc.sync.reg_load(oreg, oroi[0:1, w:w + 1])
                    off = nc.snap(oreg, min_val=0, max_val=Y1ROWS - CAP)
                    nc.sync.dma_start(
                        out=g[:],
                        in_=y1_dram[bass.ds(off, CAP), :].rearrange("(c p) d -> p c d", p=P))
                else:
                    idxf = work.tile([P, NCHUNK], F32, name="idxf")
                    nc.vector.tensor_scalar(out=idxf[:], in0=CS("chunkio"),
                                            scalar1=orepall[:, w:w + 1],
                                            scalar2=None, op0=OP.add)
                    idxi = work.tile([P, NCHUNK], I32, name="idxi")
                    nc.vector.tensor_copy(idxi[:], idxf[:])
                    for cch in range(NCHUNK):
                        nc.gpsimd.indirect_dma_start(
                            out=g[:, cch, :], out_offset=None, in_=y1_dram[:, :],
                            in_offset=bass.IndirectOffsetOnAxis(
                                ap=idxi[:, cch:cch + 1], axis=0))
                psumW = ppA.tile([P, 66], F32, tag="a", name="psumW")
                for cch in range(NCHUNK):
                    ohtw = work.tile([P, P], F32, name="ohtw")
                    nc.vector.scalar_tensor_tensor(out=ohtw[:],
                                                   in0=g[:, cch, 64:65].to_broadcast([P, P]),
                                                   scalar=float(-SEGW * w), in1=c_iotaF[:],
                                                   op0=OP.add, op1=OP.is_equal)
                    nc.tensor.matmul(psumW[:], lhsT=ohtw[:], rhs=g[:, cch, :],
                                     start=(cch == 0), stop=(cch == NCHUNK - 1))
                # ---- stage Z ----
                y2s = work.tile([P, 64], F32, name="y2s")
                nc.vector.tensor_copy(y2s[:], psumW[:, 0:64])
                ncol = work.tile([P, 1], F32, name="ncol")
                nc.vector.tensor_copy(ncol[:], psumW[:, 65:66])
                psumZt = ppT.tile([64, P], F32, tag="t", name="psumZt")
                nc.tensor.transpose(psumZt[:], y2s[:], c_ident[:])
                y2T = work.tile([64, P], F32, name="y2T")
                nc.vector.tensor_copy(y2T[:], psumZt[:])
                psumNT = ppT.tile([1, P], F32, tag="t", name="psumNT")
                nc.tensor.transpose(psumNT[:], ncol[:], c_ident[:])
                nrow = work.tile([1, P], F32, name="nrow")
                nc.vector.tensor_copy(nrow[:], psumNT[:])
                nrep = work.tile([MAXP, P], F32, name="nrep")
                nc.gpsimd.partition_broadcast(nrep[:], nrow[0:1, :])
                ohn = work.tile([MAXP, P], F32, name="ohn")
                nc.vector.tensor_tensor(out=ohn[:], in0=nrep[:],
                                        in1=c_iota17c[:, 0:1].to_broadcast([MAXP, P]),
                                        op=OP.is_equal)
                psumH = ppA.tile([P, 72], F32, tag="a", name="psumH")
                nc.tensor.matmul(psumH[:], lhsT=y2T[:], rhs=c_phiw1a[:], start=True, stop=False)
                nc.tensor.matmul(psumH[:], lhsT=ohn[:], rhs=c_phiw1b[:], start=False, stop=True)
                th1 = work.tile([P, 73], F32, name="th1")
                nc.scalar.activation(th1[:, 0:72], psumH[:], AF.Tanh)
                nc.gpsimd.memset(th1[:, 72:73], 1.0)
                psumHT = ppT.tile([73, P], F32, tag="t", name="psumHT")
                nc.tensor.transpose(psumHT[:], th1[:], c_ident[:])
                th1T = work.tile([73, P], F32, name="th1T")
                nc.vector.tensor_copy(th1T[:], psumHT[:])
                psumZ = ppA.tile([P, 64], F32, tag="a", name="psumZ")
                nc.tensor.matmul(psumZ[:], lhsT=th1T[:], rhs=c_phiw2e[:], start=True, stop=True)
                zs = work.tile([P, 64], F32, name="zs")
                nc.vector.tensor_copy(zs[:], psumZ[:])
                psumZT2 = ppT.tile([64, P], F32, tag="t", name="psumZT2")
                nc.tensor.transpose(psumZT2[:], zs[:], c_ident[:])
                nc.vector.tensor_copy(zTbuf[0:64, w * P:(w + 1) * P], psumZT2[:])
                nc.vector.tensor_copy(zTbuf16[:, w * P:(w + 1) * P], psumZT2[:])
                # size head + argmax
                psumL1 = ppA.tile([P, 40], F32, tag="a", name="psumL1")
                nc.tensor.matmul(psumL1[:], lhsT=zTbuf[:, w * P:(w + 1) * P],
                                 rhs=c_sizew1e[:], start=True, stop=True)
                th2 = work.tile([P, 41], F32, name="th2")
                nc.scalar.activation(th2[:, 0:40], psumL1[:], AF.Tanh)
                nc.gpsimd.memset(th2[:, 40:41], 1.0)
                psumT2 = ppT.tile([41, P], F32, tag="t", name="psumT2")
                nc.tensor.transpose(psumT2[:], th2[:], c_ident[:])
                th2T = work.tile([41, P], F32, name="th2T")
                nc.vector.tensor_copy(th2T[:], psumT2[:])
                psumLg = ppA.tile([P, MAXP], F32, tag="a", name="psumLg")
                nc.tensor.matmul(psumLg[:], lhsT=th2T[:], rhs=c_sizew2e[:], start=True, stop=True)
                mx = work.tile([P, 1], F32, name="mx")
                nc.vector.reduce_max(mx[:], psumLg[:], axis=AX.X)
                eqm = work.tile([P, MAXP], F32, name="eqm")
                nc.vector.tensor_scalar(out=eqm[:], in0=psumLg[:], scalar1=mx[:, 0:1],
                                        scalar2=None, op0=OP.is_equal)
                vv = work.tile([P, MAXP], F32, name="vv")
                nc.vector.tensor_tensor(out=vv[:], in0=eqm[:], in1=c_rev17[:], op=OP.mult)
                mv = work.tile([P, 1], F32, name="mv")
                nc.vector.reduce_max(mv[:], vv[:], axis=AX.X)
                ndec = work.tile([P, 1], F32, name="ndec")
                nc.vector.tensor_scalar(out=ndec[:], in0=mv[:], scalar1=-1.0, scalar2=float(MAXP),
                                        op0=OP.mult, op1=OP.add)
                nc.vector.tensor_scalar(out=mask_buf[:, w * MAXP:(w + 1) * MAXP],
                                        in0=c_iota17r[:], scalar1=ndec[:, 0:1],
                                        scalar2=None, op0=OP.is_lt)
                mrow = work.tile([P, MAXP], F32, name="mrow")
                nc.vector.tensor_copy(mrow[:], mask_buf[:, w * MAXP:(w + 1) * MAXP])
                nc.sync.dma_start(out=out_mask[w * P:(w + 1) * P, :], in_=mrow[:])

            mark('D')
            # ---- stage D: decoder (bf16 matmuls) ----
            out_v = out_xr[:, :].rearrange("(b p) d -> b p d", p=MAXP)
            groups = [(0, 4), (4, 4), (8, 4), (12, 4), (16, 1)]
            for s in range(NW):
                for (p0, G) in groups:
                    zpT4 = work.tile([64, G, P], DTD, name="zpT4")
                    nc.vector.tensor_tensor(
                        out=zpT4[:],
                        in0=zTbuf16[:, s * P:(s + 1) * P].unsqueeze(1).to_broadcast([64, G, P]),
                        in1=peDT16[:, p0:p0 + G].unsqueeze(2).to_broadcast([64, G, P]),
                        op=OP.mult)
                    psumD2 = ppA.tile([P, G * P], F32, tag="a", name="psumD2")
                    psumD14 = ppT.tile([96, G, P], F32, tag="t", name="psumD14")
                    thD4 = work.tile([97, G, P], DTD, name="thD4")
                    nc.gpsimd.memset(thD4[96:97, :, :], 1.0)
                    for pi in range(G):
                        nc.tensor.matmul(psumD14[:, pi, :], lhsT=c16_decw1[:],
                                         rhs=zpT4[:, pi, :], start=True, stop=True)
                    nc.scalar.activation(thD4[0:96, :, :], psumD14[:], AF.Tanh,
                                         bias=c_decb1c[:, 0:1])
                    for pi in range(G):
                        nc.tensor.matmul(psumD2[:, pi * P:(pi + 1) * P], lhsT=thD4[:, pi, :],
                                         rhs=c16_decw2e[:], start=True, stop=True)
                    xrs = work.tile([P, G, P], F32, name="xrs")
                    nc.vector.tensor_tensor(
                        out=xrs[:],
                        in0=psumD2[:].rearrange("p (g d) -> p g d", g=G),
                        in1=mask_buf[:, s * MAXP + p0:s * MAXP + p0 + G]
                        .unsqueeze(2).to_broadcast([P, G, P]),
                        op=OP.mult)
                    nc.sync.dma_start(out=out_v[s * P:(s + 1) * P, p0:p0 + G, :],
                                      in_=xrs[:])
    mark('end')
    nc.compile()
    return nc


def _prep_host(inputs):
    """Shard inputs + build constant tensors. Returns in_maps for 8 cores."""
    gi = lambda k: np.asarray(inputs[k], np.float32)
    x = gi("x")
    batch = np.asarray(inputs["batch"]).astype(np.int64)

    counts = np.bincount(batch, minlength=B)
    assert counts.max() <= 16, f"segment size {counts.max()} > 16 unsupported"

    cv = {}
    psi_w1 = gi("psi_w1")
    rank_w = gi("rank_w")
    cv["w1m"] = np.concatenate(
        [psi_w1, rank_w, psi_w1.mean(axis=1, keepdims=True)], axis=1)
    psi_b1 = gi("psi_b1")
    cv["bdiff"] = np.broadcast_to(psi_b1 - psi_b1.mean(), (P, 96))
    cv["psi_g"] = np.broadcast_to(gi("psi_g"), (P, 96))
    cv["psi_bt"] = np.broadcast_to(gi("psi_bt"), (P, 96))
    cv["psi_w2e"] = np.concatenate([gi("psi_w2"), gi("psi_b2")[None, :]], axis=0)
    cv["ident"] = np.eye(P, dtype=np.float32)
    cv["eye17"] = np.eye(MAXP, dtype=np.float32)
    cv["ones_row"] = np.ones((1, P), np.float32)
    cv["ones_col"] = np.ones((P, 1), np.float32)
    for t in ("e", "d"):
        cv[f"pe_{t}_w1"] = gi(f"pe_{t}_w1")
        cv[f"pe_{t}_b1"] = gi(f"pe_{t}_b1")[None, :]
        cv[f"pe_{t}_g"] = np.broadcast_to(gi(f"pe_{t}_g"), (MAXP, 40))
        cv[f"pe_{t}_bt"] = np.broadcast_to(gi(f"pe_{t}_bt"), (MAXP, 40))
        cv[f"pe_{t}_w2e"] = np.concatenate(
            [gi(f"pe_{t}_w2"), gi(f"pe_{t}_b2")[None, :]], axis=0)
    phi_w1 = gi("phi_w1")
    cv["phi_w1a"] = phi_w1[:64]
    cv["phi_w1b"] = phi_w1[64:64 + MAXP] + gi("phi_b1")[None, :]
    cv["phi_w2e"] = np.concatenate([gi("phi_w2"), gi("phi_b2")[None, :]], axis=0)
    cv["size_w1e"] = np.concatenate([gi("size_w1"), gi("size_b1")[None, :]], axis=0)
    cv["size_w2e"] = np.concatenate([gi("size_w2"), gi("size_b2")[None, :]], axis=0)
    cv["dec_b1c"] = gi("dec_b1")[:, None]
    cv["iota17c"] = np.arange(MAXP, dtype=np.float32)[:, None]
    cv["iotaF"] = np.broadcast_to(np.arange(P, dtype=np.float32), (P, P))
    cv["chunkio"] = (np.arange(P, dtype=np.float32)[:, None]
                     + P * np.arange(NCHUNK, dtype=np.float32)[None, :])
    cv["rev17"] = np.broadcast_to(
        float(MAXP) - np.arange(MAXP, dtype=np.float32), (P, MAXP))
    cv["iota17r"] = np.broadcast_to(np.arange(MAXP, dtype=np.float32), (P, MAXP))

    cpack = np.zeros((P, CPCOLS), np.float32)
    for name, r, c in _CONST_SPECS:
        v = np.asarray(cv[name], np.float32)
        assert v.shape == (r, c), f"{name}: {v.shape} != {(r, c)}"
        off = _CONST_OFF[name][0]
        cpack[:r, off:off + c] = v

    cv16 = {
        "dec_w1": gi("dec_w1"),
        "dec_w2e": np.concatenate([gi("dec_w2"), gi("dec_b2")[None, :]], axis=0),
    }
    dtd = ml_dtypes.bfloat16 if BF16_D else np.float32
    cpack16 = np.zeros((P, CP16COLS), dtd)
    for name, r, c in _CONST16_SPECS:
        v = np.asarray(cv16[name], np.float32).astype(dtd)
        assert v.shape == (r, c), f"{name}: {v.shape} != {(r, c)}"
        off = _CONST16_OFF[name][0]
        cpack16[:r, off:off + c] = v

    base = {"cpack": cpack, "cpack16": cpack16}

    bounds = np.searchsorted(batch, np.arange(0, B + 1, BLOC))
    in_maps = []
    for c in range(NCORES):
        s, e = int(bounds[c]), int(bounds[c + 1])
        nloc = e - s
        assert nloc <= NPAD, f"core {c} has {nloc} > {NPAD} elements"
        wb = np.searchsorted(batch[s:e], c * BLOC + np.arange(0, BLOC + 1, SEGW))
        assert np.diff(wb).max() <= CAP, f"window overflow on core {c}"
        xTc = np.zeros((P, NPAD), np.float32)
        xTc[:, :nloc] = x[s:e].T
        seghc = np.full(SEGH_LEN, BIGSEG, np.float32)
        seghc[HALO:HALO + nloc] = (batch[s:e] - c * BLOC).astype(np.float32)
        m = dict(base)
        m["xT"] = xTc
        m["segh"] = seghc
        in_maps.append(m)
    return in_maps


def _run(inputs, trace=False):
    if "nc" not in _CACHED:
        _CACHED["nc"] = _build_bass()
    nc = _CACHED["nc"]
    in_maps = _prep_host(inputs)
    res = run_bass_kernel_spmd(nc, in_maps, list(range(NCORES)), trace=trace)
    outs = res.results
    xr = np.concatenate([outs[c]["out_xr"] for c in range(NCORES)], axis=0)
    maskf = np.concatenate([outs[c]["out_mask"] for c in range(NCORES)], axis=0)
    mask = maskf.reshape(-1) > 0.5
    batchr = np.repeat(np.arange(B, dtype=np.int32), MAXP)
    return (xr, batchr, mask), res.exec_time_ns


def kernel(**inputs):
    out, _ = _run(inputs, trace=False)
    return out


def kernel_traced(**inputs):
    return _run(inputs, trace=True)
